# revision 4
# baseline (speedup 1.0000x reference)
"""Bass/Trainium2 kernel for nn_MultiHeadedAttention (GQA + RoPE + causal attention).

Sharding: 8 cores = 2 batch groups x 4 head-groups.
Core c: batch b=c//4, head group j=c%4 (q heads 4j..4j+3, kv head j).
Output projection is column-sharded after a 4-way AllGather of ctx^T;
host concatenates the disjoint output slices.

v2: single flat pipeline over token blocks t:
  proj(t) -> attn(qb=t) -> AllGather(qb=t) -> wo(qb=t-1)
so collectives overlap compute and the scalar-engine exp (batched over
2-tile PSUM groups) pipelines against the tensor engine.
Compute is bf16 (fp32 PSUM accumulation).
"""

import os
import sys

sys.path.insert(0, "/opt/trn_rl_repo")
import numpy as np


B, S, HID = 2, 2048, 2048
NH, NKV, D = 16, 4, 128
N_CORES = 8
GROUPS = [[0, 1, 2, 3], [4, 5, 6, 7]]
HLOC = 4          # q heads per core
TB = 512          # token block (matmul moving dim)
NTB = S // TB     # 4
HT = HID // 128   # 16 hid tiles
SCALE = float(D) ** -0.5

LAST_RESULTS = None  # stash for test harness timing


def _analyze_mask(mask):
    """Per (qblock, ktile): live pairs and mixed-mask tiles (deduped)."""
    maskb = np.asarray(mask).astype(bool)
    live = []
    mixd = {}
    uniq = []
    keys = {}
    for qb in range(NTB):
        lv = []
        for kt in range(S // 128):
            sub = maskb[qb * TB:(qb + 1) * TB, kt * 128:(kt + 1) * 128]
            if not sub.any():
                continue
            lv.append(kt)
            if sub.all():
                mixd[(qb, kt)] = None
            else:
                tile = np.ascontiguousarray(sub.T.astype(np.float32))
                kb = tile.tobytes()
                if kb not in keys:
                    keys[kb] = len(uniq)
                    uniq.append(tile)
                mixd[(qb, kt)] = keys[kb]
        live.append(lv)
    return live, mixd, uniq


def _build_program(live, mixd, n_u):
    import concourse.bass as bass  # noqa: F401
    import concourse.mybir as mybir
    from concourse import bacc, tile

    f32 = mybir.dt.float32
    bf16 = mybir.dt.bfloat16
    EXP = mybir.ActivationFunctionType.Exp

    nc = bacc.Bacc("TRN2", target_bir_lowering=False, debug=False,
                   num_devices=N_CORES)

    xT = nc.dram_tensor("xT", [HID, S], bf16, kind="ExternalInput")
    wq = nc.dram_tensor("wq", [HID, HLOC * D], bf16, kind="ExternalInput")
    wk = nc.dram_tensor("wk", [HID, D], bf16, kind="ExternalInput")
    wv = nc.dram_tensor("wv", [HID, D], bf16, kind="ExternalInput")
    wo = nc.dram_tensor("wo", [HID, TB], bf16, kind="ExternalInput")
    cosE = nc.dram_tensor("cosE", [D, S], bf16, kind="ExternalInput")
    sinP = nc.dram_tensor("sinP", [D, S], bf16, kind="ExternalInput")
    pswap = nc.dram_tensor("pswap", [128, 128], bf16, kind="ExternalInput")
    ident = nc.dram_tensor("ident", [128, 128], bf16, kind="ExternalInput")
    ones_in = nc.dram_tensor("ones_in", [128, 1], bf16, kind="ExternalInput")
    onesk1_in = nc.dram_tensor("onesk1_in", [1, 128], bf16, kind="ExternalInput")
    mmask = nc.dram_tensor("mmask", [max(n_u, 1) * 128, TB], bf16,
                           kind="ExternalInput")
    # out^T: [out_cols, tokens]; host transposes back
    out_o = nc.dram_tensor("o", [TB, S], f32, kind="ExternalOutput")

    mm = nc.tensor.matmul

    with tile.TileContext(nc, num_cores=N_CORES) as tc:
        stk0 = nc.allow_low_precision("bf16 kernel; fp32 PSUM accumulate")
        stk0.__enter__()
        with (
            tc.tile_pool(name="const", bufs=1) as cpool,
            tc.tile_pool(name="acts", bufs=1) as apool,
            tc.tile_pool(name="w", bufs=1) as wpool,
            tc.tile_pool(name="xs", bufs=32) as xpool,
            tc.tile_pool(name="exf", bufs=6) as exfpool,
            tc.tile_pool(name="st", bufs=3) as stage,
            tc.tile_pool(name="gs", bufs=6) as gpool,
            tc.tile_pool(name="ob", bufs=2) as opool,
            tc.tile_pool(name="rr", bufs=3) as rcpool,
            # PSUM: 4 + 1 + 1 + 2 = 8 banks total
            tc.tile_pool(name="pbig", bufs=2, space="PSUM") as pbig,
            tc.tile_pool(name="pcps", bufs=1, space="PSUM") as pcps,
            tc.tile_pool(name="pdps", bufs=1, space="PSUM") as pdps,
            tc.tile_pool(name="ps5", bufs=2, space="PSUM") as ps5,
            tc.tile_pool(name="dram", bufs=1, space="DRAM") as dram,
        ):
            # ---- persistent SBUF tensors ----
            qT_s = apool.tile([128, HLOC * S], bf16, tag="qT")
            kT_s = apool.tile([128, S], bf16, tag="kT")
            v_s = apool.tile([128, S], bf16, tag="v")
            ctxT_s = apool.tile([128, HLOC * S], bf16, tag="ctxT")

            # ---- input DMAs: x tiles for t=0 first, then weights ----
            xts = {}

            def load_x(t):
                for h in range(HT):
                    xt = xpool.tile([128, TB], bf16, tag="xt", name=f"xt{t}_{h}")
                    nc.sync.dma_start(
                        out=xt[:],
                        in_=xT[h * 128:(h + 1) * 128, t * TB:(t + 1) * TB])
                    xts[(t, h)] = xt

            load_x(0)

            # per-h weight tiles (fine-grained so first matmuls unblock early)
            wq_h, wk_h, wv_h = [], [], []
            for h in range(HT):
                wqt = wpool.tile([128, HLOC * D], bf16, tag=f"wq{h}",
                                 name=f"wq{h}")
                nc.sync.dma_start(out=wqt[:], in_=wq[h * 128:(h + 1) * 128, :])
                wq_h.append(wqt)
                wkt = wpool.tile([128, D], bf16, tag=f"wk{h}", name=f"wk{h}")
                nc.sync.dma_start(out=wkt[:], in_=wk[h * 128:(h + 1) * 128, :])
                wk_h.append(wkt)
                wvt = wpool.tile([128, D], bf16, tag=f"wv{h}", name=f"wv{h}")
                nc.sync.dma_start(out=wvt[:], in_=wv[h * 128:(h + 1) * 128, :])
                wv_h.append(wvt)

            ps_s = cpool.tile([128, 128], bf16, tag="ps")
            nc.sync.dma_start(out=ps_s[:], in_=pswap[:])
            id_s = cpool.tile([128, 128], bf16, tag="id")
            nc.sync.dma_start(out=id_s[:], in_=ident[:])
            cos_s = cpool.tile([D, S], bf16, tag="cos")
            nc.sync.dma_start(out=cos_s[:], in_=cosE[:])
            sin_s = cpool.tile([D, S], bf16, tag="sin")
            nc.sync.dma_start(out=sin_s[:], in_=sinP[:])
            ones_s = cpool.tile([128, 1], bf16, tag="ones")
            nc.sync.dma_start(out=ones_s[:], in_=ones_in[:])
            onesk1 = cpool.tile([1, 128], bf16, tag="onesk1")
            nc.sync.dma_start(out=onesk1[:], in_=onesk1_in[:])
            mm_s = None
            if n_u:
                mm_s = cpool.tile([128, n_u * TB], bf16, tag="mm")
                nc.sync.dma_start(
                    out=mm_s[:].rearrange("p (u n) -> p u n", n=TB),
                    in_=mmask[:].rearrange("(u p) n -> p u n", p=128),
                )

            # warm the exp table set while startup DMAs stream
            warm = stage.tile([1, 16], bf16, tag="warm")
            nc.vector.memset(warm[:], 0.0)
            warm2 = stage.tile([1, 16], bf16, tag="warm2")
            nc.scalar.activation(warm2[:], warm[:], EXP, scale=1.0)

            wo_s = wpool.tile([128, HT * TB], bf16, tag="wo")
            nc.sync.dma_start(
                out=wo_s[:].rearrange("p (h n) -> p h n", n=TB),
                in_=wo[:].rearrange("(h p) n -> p h n", p=128),
            )

            HSLOT = TB  # per-head token block inside gather tiles
            bounce = [dram.tile([128, HLOC * TB], bf16, tag=f"bn{qb}",
                                name=f"bounce{qb}") for qb in range(NTB)]
            gath = [dram.tile([HLOC * 128, HLOC * TB], bf16, tag=f"g{qb}",
                              name=f"gath{qb}") for qb in range(NTB)]

            # ================= helper: projection for token block t ========
            def proj(t):
                if t + 1 < NTB:
                    pass  # x(t+1) already loaded below after waves
                # 3 waves of 2 outputs each: (q0,q1), (q2,q3), (k,v)
                waves = [("q", 0, "q", 1), ("q", 2, "q", 3), ("k", 0, "v", 0)]
                chunks = []
                for wv_i, (k0, i0, k1, i1) in enumerate(waves):
                    ps = pbig.tile([128, 2 * TB], f32, tag="pbig",
                                   name=f"pj{t}_{wv_i}")
                    for h in range(HT):
                        st, sp = (h == 0), (h == HT - 1)
                        for sl, (kk, ii) in enumerate(((k0, i0), (k1, i1))):
                            if kk == "q":
                                w_ap = wq_h[h][:, ii * D:(ii + 1) * D]
                            elif kk == "k":
                                w_ap = wk_h[h][:]
                            else:
                                w_ap = wv_h[h][:]
                            mm(ps[:, sl * TB:(sl + 1) * TB], w_ap,
                               xts[(t, h)][:], start=st, stop=sp)
                    # evacuate + post-process
                    for sl, (kk, ii) in enumerate(((k0, i0), (k1, i1))):
                        src = ps[:, sl * TB:(sl + 1) * TB]
                        if kk == "q":
                            dst = qT_s[:, ii * S + t * TB: ii * S + (t + 1) * TB]
                            nc.vector.tensor_copy(dst, src)
                            chunks.append(dst)
                        elif kk == "k":
                            dst = kT_s[:, t * TB:(t + 1) * TB]
                            nc.vector.tensor_copy(dst, src)
                            chunks.append(dst)
                        else:
                            vstg = stage.tile([128, TB], bf16, tag="vstg")
                            nc.vector.tensor_copy(vstg[:], src)
                            for i in range(TB // 128):
                                tps = ps5.tile([128, 128], bf16, tag="s5",
                                               name="tps")
                                nc.tensor.transpose(
                                    tps[:], vstg[:, i * 128:(i + 1) * 128],
                                    id_s[:])
                                tt = t * (TB // 128) + i
                                nc.vector.tensor_copy(
                                    v_s[:, tt * 128:(tt + 1) * 128], tps[:])
                # prefetch next x block under the RoPE work
                if t + 1 < NTB:
                    load_x(t + 1)
                # RoPE on the 5 chunks (4 q heads + k)
                for ch in chunks:
                    sw = ps5.tile([128, TB], f32, tag="s5", name="swps")
                    mm(sw[:], ps_s[:], ch, start=True, stop=True)
                    swm = stage.tile([128, TB], bf16, tag="swm")
                    nc.vector.tensor_mul(swm[:], sw[:],
                                         sin_s[:, t * TB:(t + 1) * TB])
                    nc.vector.tensor_mul(ch, ch, cos_s[:, t * TB:(t + 1) * TB])
                    nc.vector.tensor_add(ch, ch, swm[:])

            # ================= helper: attention for q block qb ============
            def attn(qb):
                lv = live[qb]
                for h in range(HLOC):
                    qslice = qT_s[:, h * S + qb * TB: h * S + (qb + 1) * TB]
                    groups = [lv[i:i + 2] for i in range(0, len(lv), 2)]
                    nquad = (len(lv) + 3) // 4
                    cps = pcps.tile([128, TB], f32, tag="cps",
                                    name=f"cps{qb}_{h}")
                    dps = pdps.tile([1, TB], f32, tag="dps",
                                    name=f"dps{qb}_{h}")
                    exs = []     # (ex_tile, kts) pending AV
                    pend = []    # pending quad partial-sum tiles

                    def do_av(gi):
                        ex, kts = exs[gi]
                        first = (gi == 0)
                        last = (gi == len(groups) - 1)
                        for i, kt in enumerate(kts):
                            nc_start = first and i == 0
                            nc_stop = last and i == len(kts) - 1
                            mm(cps[:], v_s[:, kt * 128:(kt + 1) * 128],
                               ex[:, i * TB:(i + 1) * TB],
                               start=nc_start, stop=nc_stop)
                        # denominator partial: sum the (up to) 2 slices
                        if len(kts) == 2:
                            pa = exfpool.tile([128, TB], bf16, tag="ex",
                                              name="pa")
                            nc.vector.tensor_add(pa[:], ex[:, 0:TB],
                                                 ex[:, TB:2 * TB])
                        else:
                            pa = ex  # single slice
                        pend.append(pa)
                        if len(pend) == 2:
                            pq = exfpool.tile([128, TB], bf16, tag="ex",
                                              name="pq")
                            nc.vector.tensor_add(pq[:], pend[0][:, 0:TB],
                                                 pend[1][:, 0:TB])
                            pend[:] = [pq]
                        qi = gi // 2
                        if gi % 2 == 1 or last:
                            mm(dps[:], ones_s[:], pend[0][:, 0:TB],
                               start=(qi == 0), stop=(qi == nquad - 1))
                            pend[:] = []

                    for gi, kts in enumerate(groups):
                        sps = pbig.tile([128, 2 * TB], f32, tag="pbig",
                                        name=f"sps{qb}_{h}_{gi}")
                        for i, kt in enumerate(kts):
                            mm(sps[:, i * TB:(i + 1) * TB],
                               kT_s[:, kt * 128:(kt + 1) * 128], qslice,
                               start=True, stop=True)
                        ex = exfpool.tile([128, 2 * TB], bf16, tag="ex",
                                          name=f"ex{qb}_{h}_{gi}")
                        nc.scalar.activation(ex[:, 0:len(kts) * TB],
                                             sps[:, 0:len(kts) * TB], EXP,
                                             scale=SCALE)
                        for i, kt in enumerate(kts):
                            u = mixd[(qb, kt)]
                            if u is not None:
                                nc.vector.tensor_mul(
                                    ex[:, i * TB:(i + 1) * TB],
                                    ex[:, i * TB:(i + 1) * TB],
                                    mm_s[:, u * TB:(u + 1) * TB])
                        exs.append((ex, kts))
                        if gi > 0:
                            do_av(gi - 1)
                    do_av(len(groups) - 1)

                    # normalization
                    rc = rcpool.tile([1, TB], f32, tag="rc")
                    nc.vector.reciprocal_approx_fast(rc[:], dps[:])
                    rcb = rcpool.tile([1, TB], bf16, tag="rcb")
                    nc.vector.tensor_copy(rcb[:], rc[:])
                    bps = ps5.tile([128, TB], f32, tag="s5", name="bps")
                    mm(bps[:], onesk1[:], rcb[:], start=True, stop=True)
                    bcs = stage.tile([128, TB], bf16, tag="bcs")
                    nc.vector.tensor_copy(bcs[:], bps[:])
                    nc.vector.tensor_mul(
                        ctxT_s[:, h * S + qb * TB: h * S + (qb + 1) * TB],
                        cps[:], bcs[:])

            # ================= helper: gather launch for q block qb ========
            def gather(qb):
                for h in range(HLOC):
                    nc.sync.dma_start(
                        out=bounce[qb][:, h * HSLOT:(h + 1) * HSLOT],
                        in_=ctxT_s[:, h * S + qb * TB: h * S + (qb + 1) * TB])
                import concourse.mybir as mybir
                nc.gpsimd.collective_compute(
                    "AllGather",
                    mybir.AluOpType.bypass,
                    replica_groups=GROUPS,
                    ins=[bounce[qb].opt()],
                    outs=[gath[qb].opt()],
                )

            # ================= helper: output projection for q block qb ====
            def wo_proj(qb):
                # out^T accumulators: 4 col-tiles across two 2-bank psum tiles
                opsA = pbig.tile([128, 2 * TB], f32, tag="pbig",
                                 name=f"woA{qb}")
                opsB = pbig.tile([128, 2 * TB], f32, tag="pbig",
                                 name=f"woB{qb}")

                def ops_slice(ct):
                    ps = opsA if ct < 2 else opsB
                    c = ct % 2
                    return ps[:, c * TB:(c + 1) * TB]

                # (j core, h local) -> g = 4j+h; each gs tile freed after use
                for j in range(HLOC):
                    for h in range(HLOC):
                        g = 4 * j + h
                        gt = gpool.tile([128, TB], bf16, tag="gs",
                                        name=f"gs{qb}_{g}")
                        nc.sync.dma_start(
                            out=gt[:],
                            in_=gath[qb][j * 128:(j + 1) * 128,
                                         h * HSLOT:(h + 1) * HSLOT])
                        for ct in range(TB // 128):
                            mm(ops_slice(ct),
                               wo_s[:, g * TB + ct * 128:
                                    g * TB + (ct + 1) * 128],
                               gt[:], start=(g == 0), stop=(g == HT - 1))
                for ct in range(TB // 128):
                    osb = opool.tile([128, TB], f32, tag="osb")
                    nc.vector.tensor_copy(osb[:], ops_slice(ct))
                    nc.sync.dma_start(
                        out=out_o[ct * 128:(ct + 1) * 128,
                                  qb * TB:(qb + 1) * TB],
                        in_=osb[:])

            # ================= the pipeline ================================
            for t in range(NTB):
                proj(t)
                attn(t)
                gather(t)
                if t >= 1:
                    wo_proj(t - 1)
            wo_proj(NTB - 1)

        stk0.__exit__(None, None, None)
    nc.compile()
    return nc


def kernel(x, wq, wk, wv, wo, cos, sin, mask):
    global LAST_RESULTS
    import ml_dtypes
    from concourse.bass_utils import run_bass_kernel_spmd

    bfnp = ml_dtypes.bfloat16
    x = np.asarray(x, np.float32)
    wq = np.asarray(wq, np.float32)
    wk = np.asarray(wk, np.float32)
    wv = np.asarray(wv, np.float32)
    wo = np.asarray(wo, np.float32)
    cos = np.asarray(cos, np.float32)
    sin = np.asarray(sin, np.float32)

    live, mixd, uniq = _analyze_mask(mask)
    n_u = len(uniq)
    mmask = (np.concatenate(uniq, axis=0) if n_u
             else np.zeros((128, TB), np.float32))

    cosE = np.repeat(cos, 2, axis=1).T
    sp = np.repeat(sin, 2, axis=1).copy()
    sp[:, 0::2] *= -1.0
    sinP = sp.T
    pswap = np.zeros((128, 128), np.float32)
    pswap[np.arange(128), np.arange(128) ^ 1] = 1.0
    ident = np.eye(128, dtype=np.float32)

    nc = _build_program(live, mixd, n_u)

    def b(a):
        return np.ascontiguousarray(np.asarray(a).astype(bfnp))

    in_maps = []
    for c in range(N_CORES):
        bb, j = c // 4, c % 4
        in_maps.append({
            "xT": b(x[bb].T),
            "wq": b(wq[:, 512 * j:512 * (j + 1)]),
            "wk": b(wk[:, 128 * j:128 * (j + 1)]),
            "wv": b(wv[:, 128 * j:128 * (j + 1)]),
            "wo": b(wo[:, 512 * j:512 * (j + 1)]),
            "cosE": b(cosE), "sinP": b(sinP), "pswap": b(pswap),
            "ident": b(ident),
            "ones_in": b(np.ones((128, 1), np.float32)),
            "onesk1_in": b(np.ones((1, 128), np.float32)),
            "mmask": b(mmask),
        })

    res = run_bass_kernel_spmd(nc, in_maps, list(range(N_CORES)))
    LAST_RESULTS = res

    out = np.empty((B, S, HID), np.float32)
    for c in range(N_CORES):
        bb, j = c // 4, c % 4
        out[bb, :, 512 * j:512 * (j + 1)] = res.results[c]["o"].T
    return out
